# revision 5
# baseline (speedup 1.0000x reference)
"""Embedding lookup (nn_CustomEmbedding) on 8 Trainium2 NeuronCores.

reference: out[b, t, :] = weight.T[index[b, t], :]
  index:  [4096, 200] int32  (values in [0, 100000))
  weight: [128, 100000] f32
  out:    [4096, 200, 128] f32

Strategy (data-parallel batch shard, replicated table):
  - Host: table = weight.T -> [V=100000, D=128] contiguous (512B rows).
  - Shard the 819200 flat lookups across 8 cores (102400 each).
  - Per core: load indices into SBUF once, then stream blocks of 128*K
    lookups: K indirect-DMA gathers (128 rows each, one offset per
    partition -- the only offset shape walrus unrolls correctly) fill a
    [128, K*D] tile, followed by one fully-regular 128-partition store
    (K*512B contiguous per partition).
  - Index layout is host-permuted (a pure C-order reshape) so partition p
    of block b holds output rows (b*128 + p)*K .. +K-1.

Perf note: this schedule is SWDGE-descriptor-rate-bound. The GpSimd Q7
generates gather descriptors at ~8.6ns each (measured; both for
indirect_dma_start and InstDMAGatherAnt), plus ~310ns sequencer pitch per
instruction => 102400 descs/core ~= 1.15ms. Schemes that cut instruction
count via int16 dma_gather windows (vocab parity split etc.) add dummy
descriptors that cost the same 8ns each and measure strictly worse.
"""

import numpy as np

import concourse.bacc as bacc
import concourse.mybir as mybir
import concourse.tile as tile
from concourse import bass
from concourse.bass_utils import run_bass_kernel_spmd

V = 100000
D = 128
N_CORES = 8
N_TOTAL = 4096 * 200  # 819200
N_CORE = N_TOTAL // N_CORES  # 102400 = 128 * 800
COLS = N_CORE // 128  # 800 index columns per partition
K = 50  # lookups per partition per block
NB = COLS // K  # 16 blocks
GATHER_BUFS = 4

_cached = {}


def _build():
    nc = bacc.Bacc(
        "TRN2",
        target_bir_lowering=False,
        debug=False,
        enable_asserts=False,
        num_devices=N_CORES,
    )
    idx_dram = nc.dram_tensor("idx", [128, COLS], mybir.dt.int32, kind="ExternalInput")
    table_dram = nc.dram_tensor("table", [V, D], mybir.dt.float32, kind="ExternalInput")
    out_dram = nc.dram_tensor(
        "out", [N_CORE, D], mybir.dt.float32, kind="ExternalOutput"
    )

    # out viewed as [NB, 128, K*D]: block b, partition p covers rows
    # (b*128 + p)*K .. +K-1 -- contiguous K*D elements per partition.
    out_r = out_dram.ap().rearrange("(nb p k) d -> nb p (k d)", p=128, k=K)

    with tile.TileContext(nc) as tc:
        with (
            tc.tile_pool(name="idxp", bufs=1) as idx_pool,
            tc.tile_pool(name="gp", bufs=GATHER_BUFS) as gpool,
        ):
            idx_tile = idx_pool.tile([128, COLS], mybir.dt.int32)
            nc.sync.dma_start(idx_tile[:], idx_dram.ap())
            for b in range(NB):
                gtile = gpool.tile([128, K * D], mybir.dt.float32)
                for j in range(K):
                    c = b * K + j
                    # one offset per partition (the only HW-supported shape):
                    # partition p gets table[idx[p, c], :] -> gtile[p, j*D:(j+1)*D]
                    nc.gpsimd.indirect_dma_start(
                        out=gtile[:, j * D : (j + 1) * D],
                        out_offset=None,
                        in_=table_dram.ap(),
                        in_offset=bass.IndirectOffsetOnAxis(
                            ap=idx_tile[:, c : c + 1], axis=0
                        ),
                    )
                nc.sync.dma_start(out_r[b], gtile[:])
    nc.compile()
    return nc


def _get_nc():
    if "nc" not in _cached:
        _cached["nc"] = _build()
    return _cached["nc"]


def make_in_maps(index: np.ndarray, weight: np.ndarray):
    idx_flat = np.ascontiguousarray(index, dtype=np.int32).reshape(-1)
    table = np.ascontiguousarray(weight.T.astype(np.float32, copy=False))

    in_maps = []
    for c in range(N_CORES):
        chunk = idx_flat[c * N_CORE : (c + 1) * N_CORE]
        # [NB, 128, K] -> [128, NB*K]: partition p, col b*K+j = flat row
        # b*128*K + p*K + j of this core's chunk.
        arranged = np.ascontiguousarray(
            chunk.reshape(NB, 128, K).transpose(1, 0, 2).reshape(128, COLS)
        )
        in_maps.append({"idx": arranged, "table": table})
    return in_maps


def kernel(index: np.ndarray, weight: np.ndarray) -> np.ndarray:
    in_maps = make_in_maps(index, weight)
    nc = _get_nc()
    res = run_bass_kernel_spmd(nc, in_maps, core_ids=list(range(N_CORES)))
    outs = [r["out"] for r in res.results]
    full = np.concatenate(outs, axis=0)  # [819200, 128]
    return full.reshape(index.shape[0], index.shape[1], D)


# revision 6
# speedup vs baseline: 1.0880x; 1.0880x over previous
"""Embedding lookup (nn_CustomEmbedding) on 8 Trainium2 NeuronCores.

reference: out[b, t, :] = weight.T[index[b, t], :]
  index:  [4096, 200] int32  (values in [0, 100000))
  weight: [128, 100000] f32
  out:    [4096, 200, 128] f32

Strategy (data-parallel batch shard, replicated table, 2-pass parity
dma_gather on two SWDGE queues):
  - Host: ext = [weight.T ; zero rows] -> [100130, D] contiguous 512B rows.
  - Shard the 819200 flat lookups across 8 cores (102400 each).
  - Gathers run as hardware-looped `dma_gather` (int16 indices). int16 spans
    only 65536 row-addresses, so the table is covered in TWO passes with a
    2-row (1KB) stride and a signed mid-window base:
      pass E: base row 65536, idx=(v>>1)-32768 addresses all EVEN rows
      pass O: base row 65537, same idx formula addresses all ODD rows
    Slots whose lookup has the other parity read one of 64 spread-out zero
    rows appended after the table; dst_E + dst_O (DVE add) merges the passes.
  - The two passes are issued on DIFFERENT SWDGE queues (queue_num 0/1,
    num_swdge_queues=2): their Q7 descriptor generation overlaps (~1.7x),
    which is what beats the one-desc-per-lookup indirect-DMA schedule.
  - Slot layout is chosen so every store is a fully regular 128-partition
    DMA with NCOLS*512B contiguous bytes per partition.

Measured (8 cores, NTFF): ~1.04ms, bit-exact. Pool desc-gen remains the
bottleneck: SWDGE generates data-dependent descriptors at ~8.6ns each
serial, ~4.7ns effective with two queues; 2 passes x 102432 descs/core.
(The indirect_dma_start alternative — one desc per lookup, no dummies —
measures 1.15ms and cannot use queue parallelism: walrus pins InstDMACopy
to queue 0 regardless of the BIR queue field.)
"""

import numpy as np

import concourse.bacc as bacc
import concourse.mybir as mybir
import concourse.tile as tile
from concourse.bass_utils import run_bass_kernel_spmd

V = 100000
D = 128
EXT = 100130  # V vocab rows + 130 zero rows (64-way spread dummies + slack)
N_CORES = 8
N_TOTAL = 4096 * 200  # 819200
N_CORE = N_TOTAL // N_CORES  # 102400
NI = 6400  # lookups per gather instruction (before pad)
NIP = NI + 16  # +16 trailing always-positive dummies (defeats per-lane
#                trailing-negative truncation in the gather ucode)
NCOLS = NI // 128  # 50 columns of gathered rows per partition
NG = N_CORE // NI  # 16 groups
ICOLS = NIP // 16  # 401 int16 index columns in the 16-partition stripe
DUMMY_BASE = 17232  # idx of first zero row under the (v>>1)-32768 mapping

_cached = {}


def _build():
    nc = bacc.Bacc(
        "TRN2",
        target_bir_lowering=False,
        debug=False,
        enable_asserts=False,
        num_devices=N_CORES,
        num_swdge_queues=2,
    )
    idxE_dram = nc.dram_tensor(
        "idxE", [128, NG * ICOLS], mybir.dt.int16, kind="ExternalInput"
    )
    idxO_dram = nc.dram_tensor(
        "idxO", [128, NG * ICOLS], mybir.dt.int16, kind="ExternalInput"
    )
    ext_dram = nc.dram_tensor("ext", [EXT, D], mybir.dt.float32, kind="ExternalInput")
    out_dram = nc.dram_tensor(
        "out", [N_CORE, D], mybir.dt.float32, kind="ExternalOutput"
    )

    # even/odd row views with a 2-row (256-element) stride, based mid-window
    # so signed int16 indices reach the whole table
    even_view = (
        ext_dram.ap()[65536 : 65536 + 34592]
        .rearrange("(a two) d -> a two d", two=2)[:, 0, :]
    )
    odd_view = (
        ext_dram.ap()[65537 : 65537 + 34592]
        .rearrange("(a two) d -> a two d", two=2)[:, 0, :]
    )

    # out viewed as [NG, 128, NCOLS*D]: group g, partition p holds rows
    # g*NI + p*NCOLS .. +NCOLS-1 -- contiguous NCOLS*D elements.
    out_r = out_dram.ap().rearrange("(g p c) d -> g p (c d)", p=128, c=NCOLS)

    with tile.TileContext(nc) as tc:
        with (
            tc.tile_pool(name="idxp", bufs=1) as idx_pool,
            tc.tile_pool(name="ge", bufs=2) as gpool_e,
            tc.tile_pool(name="go", bufs=2) as gpool_o,
        ):
            idxE_tile = idx_pool.tile([128, NG * ICOLS], mybir.dt.int16)
            idxO_tile = idx_pool.tile([128, NG * ICOLS], mybir.dt.int16)
            nc.sync.dma_start(idxE_tile[:], idxE_dram.ap())
            nc.sync.dma_start(idxO_tile[:], idxO_dram.ap())
            for g in range(NG):
                dstE = gpool_e.tile([128, (NCOLS + 1) * D], mybir.dt.float32)
                dstO = gpool_o.tile([128, (NCOLS + 1) * D], mybir.dt.float32)
                nc.gpsimd.dma_gather(
                    out_ap=dstE[:].rearrange("p (c d) -> p c d", d=D),
                    in_ap=even_view,
                    idxs_ap=idxE_tile[:, g * ICOLS : (g + 1) * ICOLS],
                    num_idxs=NIP,
                    num_idxs_reg=NIP,
                    elem_size=D,
                    elem_step=2 * D,
                    single_packet=False,
                    queue_num=0,
                )
                nc.gpsimd.dma_gather(
                    out_ap=dstO[:].rearrange("p (c d) -> p c d", d=D),
                    in_ap=odd_view,
                    idxs_ap=idxO_tile[:, g * ICOLS : (g + 1) * ICOLS],
                    num_idxs=NIP,
                    num_idxs_reg=NIP,
                    elem_size=D,
                    elem_step=2 * D,
                    single_packet=False,
                    queue_num=1,
                )
                nc.vector.tensor_add(
                    out=dstE[:, : NCOLS * D],
                    in0=dstE[:, : NCOLS * D],
                    in1=dstO[:, : NCOLS * D],
                )
                nc.sync.dma_start(out_r[g], dstE[:, : NCOLS * D])
    nc.compile()
    return nc


def _get_nc():
    if "nc" not in _cached:
        _cached["nc"] = _build()
    return _cached["nc"]


# slot i (gather list position) <-> within-group position t: the gather
# writes entry i to dst[i % 128, i // 128], and partition p must hold
# positions p*NCOLS .. +NCOLS-1, so i = (t % NCOLS)*128 + (t // NCOLS).
_T_OF_SLOT = np.arange(NI).reshape(128, NCOLS).T.ravel()  # slot i -> t
_DUMMY = (DUMMY_BASE + (np.arange(NIP) & 63)).astype(np.int16)  # per-slot zero row


def _arrange_pass(vals: np.ndarray, keep: np.ndarray) -> np.ndarray:
    """Build the [128, NG*ICOLS] int16 index tensor for one parity pass.

    vals: int16 [N_CORE] gather index per position ((v>>1) - 32768)
    keep: bool [N_CORE] whether this position belongs to this pass
    The [16, ICOLS] stripe (entry i at [i%16, i//16]) is replicated 8x down
    the partitions -- one copy per GpSimd Q7 core.
    """
    out = np.empty((128, NG * ICOLS), dtype=np.int16)
    for g in range(NG):
        v_g = vals[g * NI : (g + 1) * NI]
        k_g = keep[g * NI : (g + 1) * NI]
        slots = _DUMMY.copy()
        slots[:NI][...] = np.where(k_g[_T_OF_SLOT], v_g[_T_OF_SLOT], _DUMMY[:NI])
        stripe = slots.reshape(ICOLS, 16).T  # [16, ICOLS]
        out[:, g * ICOLS : (g + 1) * ICOLS] = np.tile(stripe, (8, 1))
    return out


def make_in_maps(index: np.ndarray, weight: np.ndarray):
    idx_flat = np.ascontiguousarray(index, dtype=np.int64).reshape(-1)
    table = weight.T.astype(np.float32, copy=False)
    ext = np.zeros((EXT, D), dtype=np.float32)
    ext[:V] = table

    in_maps = []
    for c in range(N_CORES):
        v = idx_flat[c * N_CORE : (c + 1) * N_CORE]
        base = ((v >> 1) - 32768).astype(np.int16)
        even = (v & 1) == 0
        in_maps.append(
            {
                "idxE": _arrange_pass(base, even),
                "idxO": _arrange_pass(base, ~even),
                "ext": ext,
            }
        )
    return in_maps


def kernel(index: np.ndarray, weight: np.ndarray) -> np.ndarray:
    in_maps = make_in_maps(index, weight)
    nc = _get_nc()
    res = run_bass_kernel_spmd(nc, in_maps, core_ids=list(range(N_CORES)))
    outs = [r["out"] for r in res.results]
    full = np.concatenate(outs, axis=0)  # [819200, 128]
    return full.reshape(index.shape[0], index.shape[1], D)


# revision 8
# speedup vs baseline: 1.2951x; 1.1903x over previous
"""Embedding lookup (nn_CustomEmbedding) on 8 Trainium2 NeuronCores.

reference: out[b, t, :] = weight.T[index[b, t], :]
  index:  [4096, 200] int32  (values in [0, 100000))
  weight: [128, 100000] f32
  out:    [4096, 200, 128] f32

Strategy (data-parallel batch shard, replicated table, 2-pass parity
dma_gather on two SWDGE queues):
  - Host: ext = [weight.T ; zero rows] -> [100130, D] contiguous 512B rows.
  - Shard the 819200 flat lookups across 8 cores (102400 each).
  - Gathers run as hardware-looped `dma_gather` (int16 indices). int16 spans
    only 65536 row-addresses, so the table is covered in TWO passes with a
    2-row (1KB) stride and a signed mid-window base:
      pass E: base row 65536, idx=(v>>1)-32768 addresses all EVEN rows
      pass O: base row 65537, same idx formula addresses all ODD rows
    Slots whose lookup has the other parity read one of 64 spread-out zero
    rows appended after the table; dst_E + dst_O (DVE add) merges the passes.
  - The two passes are issued on DIFFERENT SWDGE queues (queue_num 0/1,
    num_swdge_queues=2): their Q7 descriptor generation overlaps (~1.7x),
    which is what beats the one-desc-per-lookup indirect-DMA schedule.
  - Slot layout is chosen so every store is a fully regular 128-partition
    DMA with NCOLS*512B contiguous bytes per partition.

Measured (8 cores, NTFF): ~1.04ms, bit-exact. Pool desc-gen remains the
bottleneck: SWDGE generates data-dependent descriptors at ~8.6ns each
serial, ~4.7ns effective with two queues; 2 passes x 102432 descs/core.
(The indirect_dma_start alternative — one desc per lookup, no dummies —
measures 1.15ms and cannot use queue parallelism: walrus pins InstDMACopy
to queue 0 regardless of the BIR queue field.)
"""

import numpy as np

import concourse.bacc as bacc
import concourse.mybir as mybir
import concourse.tile as tile
from concourse.bass_utils import run_bass_kernel_spmd

V = 100000
D = 128
EXT = 100130  # V vocab rows + 130 zero rows (64-way spread dummies + slack)
N_CORES = 8
N_TOTAL = 4096 * 200  # 819200
N_CORE = N_TOTAL // N_CORES  # 102400
NI = 3200  # lookups per gather instruction (before pad)
NIP = NI + 16  # +16 trailing always-positive dummies (defeats per-lane
#                trailing-negative truncation in the gather ucode)
NCOLS = NI // 128  # 50 columns of gathered rows per partition
NG = N_CORE // NI  # 16 groups
ICOLS = NIP // 16  # 401 int16 index columns in the 16-partition stripe
DUMMY_BASE = 17232  # idx of first zero row under the (v>>1)-32768 mapping

_cached = {}


def _build():
    nc = bacc.Bacc(
        "TRN2",
        target_bir_lowering=False,
        debug=False,
        enable_asserts=False,
        num_devices=N_CORES,
        num_swdge_queues=2,
    )
    idxE_dram = nc.dram_tensor(
        "idxE", [128, NG * ICOLS], mybir.dt.int16, kind="ExternalInput"
    )
    idxO_dram = nc.dram_tensor(
        "idxO", [128, NG * ICOLS], mybir.dt.int16, kind="ExternalInput"
    )
    ext_dram = nc.dram_tensor("ext", [EXT, D], mybir.dt.float32, kind="ExternalInput")
    out_dram = nc.dram_tensor(
        "out", [N_CORE, D], mybir.dt.float32, kind="ExternalOutput"
    )

    # even/odd row views with a 2-row (256-element) stride, based mid-window
    # so signed int16 indices reach the whole table
    even_view = (
        ext_dram.ap()[65536 : 65536 + 34592]
        .rearrange("(a two) d -> a two d", two=2)[:, 0, :]
    )
    odd_view = (
        ext_dram.ap()[65537 : 65537 + 34592]
        .rearrange("(a two) d -> a two d", two=2)[:, 0, :]
    )

    # out viewed as [NG, 128, NCOLS*D]: group g, partition p holds rows
    # g*NI + p*NCOLS .. +NCOLS-1 -- contiguous NCOLS*D elements.
    out_r = out_dram.ap().rearrange("(g p c) d -> g p (c d)", p=128, c=NCOLS)

    with tile.TileContext(nc) as tc:
        with (
            tc.tile_pool(name="idxp", bufs=1) as idx_pool,
            tc.tile_pool(name="ge", bufs=3) as gpool_e,
            tc.tile_pool(name="go", bufs=3) as gpool_o,
        ):
            idxE_tile = idx_pool.tile([128, NG * ICOLS], mybir.dt.int16)
            idxO_tile = idx_pool.tile([128, NG * ICOLS], mybir.dt.int16)
            nc.sync.dma_start(idxE_tile[:], idxE_dram.ap())
            nc.sync.dma_start(idxO_tile[:], idxO_dram.ap())
            for g in range(NG):
                dstE = gpool_e.tile([128, (NCOLS + 1) * D], mybir.dt.float32)
                dstO = gpool_o.tile([128, (NCOLS + 1) * D], mybir.dt.float32)
                nc.gpsimd.dma_gather(
                    out_ap=dstE[:].rearrange("p (c d) -> p c d", d=D),
                    in_ap=even_view,
                    idxs_ap=idxE_tile[:, g * ICOLS : (g + 1) * ICOLS],
                    num_idxs=NIP,
                    num_idxs_reg=NIP,
                    elem_size=D,
                    elem_step=2 * D,
                    single_packet=False,
                    queue_num=0,
                )
                nc.gpsimd.dma_gather(
                    out_ap=dstO[:].rearrange("p (c d) -> p c d", d=D),
                    in_ap=odd_view,
                    idxs_ap=idxO_tile[:, g * ICOLS : (g + 1) * ICOLS],
                    num_idxs=NIP,
                    num_idxs_reg=NIP,
                    elem_size=D,
                    elem_step=2 * D,
                    single_packet=False,
                    queue_num=1,
                )
                nc.vector.tensor_add(
                    out=dstE[:, : NCOLS * D],
                    in0=dstE[:, : NCOLS * D],
                    in1=dstO[:, : NCOLS * D],
                )
                nc.sync.dma_start(out_r[g], dstE[:, : NCOLS * D])
    nc.compile()
    return nc


def _get_nc():
    if "nc" not in _cached:
        _cached["nc"] = _build()
    return _cached["nc"]


# slot i (gather list position) <-> within-group position t: the gather
# writes entry i to dst[i % 128, i // 128], and partition p must hold
# positions p*NCOLS .. +NCOLS-1, so i = (t % NCOLS)*128 + (t // NCOLS).
_T_OF_SLOT = np.arange(NI).reshape(128, NCOLS).T.ravel()  # slot i -> t
_DUMMY = (DUMMY_BASE + (np.arange(NIP) & 63)).astype(np.int16)  # per-slot zero row


def _arrange_pass(vals: np.ndarray, keep: np.ndarray) -> np.ndarray:
    """Build the [128, NG*ICOLS] int16 index tensor for one parity pass.

    vals: int16 [N_CORE] gather index per position ((v>>1) - 32768)
    keep: bool [N_CORE] whether this position belongs to this pass
    The [16, ICOLS] stripe (entry i at [i%16, i//16]) is replicated 8x down
    the partitions -- one copy per GpSimd Q7 core.
    """
    out = np.empty((128, NG * ICOLS), dtype=np.int16)
    for g in range(NG):
        v_g = vals[g * NI : (g + 1) * NI]
        k_g = keep[g * NI : (g + 1) * NI]
        slots = _DUMMY.copy()
        slots[:NI][...] = np.where(k_g[_T_OF_SLOT], v_g[_T_OF_SLOT], _DUMMY[:NI])
        stripe = slots.reshape(ICOLS, 16).T  # [16, ICOLS]
        out[:, g * ICOLS : (g + 1) * ICOLS] = np.tile(stripe, (8, 1))
    return out


def make_in_maps(index: np.ndarray, weight: np.ndarray):
    idx_flat = np.ascontiguousarray(index, dtype=np.int64).reshape(-1)
    table = weight.T.astype(np.float32, copy=False)
    ext = np.zeros((EXT, D), dtype=np.float32)
    ext[:V] = table

    in_maps = []
    for c in range(N_CORES):
        v = idx_flat[c * N_CORE : (c + 1) * N_CORE]
        base = ((v >> 1) - 32768).astype(np.int16)
        even = (v & 1) == 0
        in_maps.append(
            {
                "idxE": _arrange_pass(base, even),
                "idxO": _arrange_pass(base, ~even),
                "ext": ext,
            }
        )
    return in_maps


def kernel(index: np.ndarray, weight: np.ndarray) -> np.ndarray:
    in_maps = make_in_maps(index, weight)
    nc = _get_nc()
    res = run_bass_kernel_spmd(nc, in_maps, core_ids=list(range(N_CORES)))
    outs = [r["out"] for r in res.results]
    full = np.concatenate(outs, axis=0)  # [819200, 128]
    return full.reshape(index.shape[0], index.shape[1], D)
